# revision 16
# baseline (speedup 1.0000x reference)
"""Trainium2 Bass kernel for windowed multi-head attention with relative
position bias (Swin-style block):

    qkv = x @ qkv_w.T + [q_bias, 0, v_bias]
    q, k, v = split(qkv);  q *= hd**-0.5
    attn = softmax(q @ k.T + rel_table[rel_index])
    out  = (attn @ v) @ proj_w.T + proj_b

Shapes: x [8, 32, 32, 768], 12 heads, head_dim 64, N=1024 tokens.

Sharding: pure data-parallel - one batch element per NeuronCore, 8 cores,
no collectives.

Design notes (V15; V14 notes still apply where unchanged):
  - all matmuls fp16 (weights/x cast on host); psum accumulates f32.
  - S psum is two [128,512] tiles (1 bank each, ring of 2) instead of one
    [128,1024]: the exp for half kt reads bank A while the next S matmul
    writes bank B, removing the per-kt PE stall on the exp freeing the
    bank (V14's ~0.7-1.5us/kt gap that also dropped the PE p-state).
    exp runs per 512-half on ACT; back-to-back ACT throughput is
    ap*0.833ns (latency pipelines away), so ACT does not pace.
  - denominator chain: DVE copies the s row [1,1024] out of PSUM (ACT ln
    on 1 partition cost 853ns; DVE has slack), DMA round-trip reshapes to
    [128,8], ln+exp(-x) on ACT cost ~190ns each, DMA out + stride-0
    broadcast read to [64,1024], then the fused STT normalize as in V14.
    The chain for head h-2 is WOVEN into segment h (parts after kt slots
    0/2/4) so the in-order ACT queue never blocks segment exps behind the
    chain's DMA waits, and po[h-2] frees mid-segment.
  - proj is progressive: k0..3 contributions accumulate into an SBUF f32
    acc (+proj bias) as stream fillers during segments 10-11 (their attn
    c-tiles are normalized segments earlier); tail does per-j (k4,k5)
    psum groups (k4 during chain(11)'s DMA latency, k5 after STT(11)),
    final evict = STT(psum*1 + acc).
  - engine rule (V14): gpsimd (Pool) only holds work with early-ready
    inputs: kt 6/7 bias-multiplies ([128,1024] granularity); head 11's
    kt7 goes to DVE (no next segment hides it).
  - PSUM: pss [128,512]x2 + po [65,2x512]x2 + shared qkv/proj pool
    [128,512]x2 = 8 banks exactly.
"""

import numpy as np

_CACHE = {}

B = 8
WS = 32
N = WS * WS            # 1024 tokens
C = 768
NH = 12
HD = 64
P = 128
QC = 2                 # q chunks of 512
QN = N // QC           # 512
KT = N // P            # 8 k tiles
CT = C // P            # 6 contraction tiles
OT_QK = (2 * C) // P   # 12 output tiles for q,k rows
VC = 2                 # v output chunks of 384
VN = C // VC           # 384
NP = NH // 2           # 6 head pairs
RT = N // P            # 8 = columns of the [128,8] reshaped denominators


def _build():
    import concourse.bass as bass
    import concourse.bacc as bacc
    import concourse.mybir as mybir
    import concourse.tile as tile
    import itertools
    from concourse.bass import _add_dep_helper

    f32 = mybir.dt.float32
    f16 = mybir.dt.float16
    AF = mybir.ActivationFunctionType
    MUL = mybir.AluOpType.mult
    ADD = mybir.AluOpType.add

    nc = bacc.Bacc(None, target_bir_lowering=False)

    xT_d = nc.dram_tensor("xT", [C, N], f16, kind="ExternalInput")
    wqk_d = nc.dram_tensor("wqk", [C, 2 * C], f16, kind="ExternalInput")
    wv_d = nc.dram_tensor("wv", [C, C], f16, kind="ExternalInput")
    wp_d = nc.dram_tensor("wp", [C, C], f16, kind="ExternalInput")
    qkb_d = nc.dram_tensor("qkb", [OT_QK, P], f32, kind="ExternalInput")
    vb_d = nc.dram_tensor("vb", [C], f16, kind="ExternalInput")
    pb_d = nc.dram_tensor("pb", [CT, P], f32, kind="ExternalInput")
    biasT_d = nc.dram_tensor("biasT", [NH, N, N], f16, kind="ExternalInput")
    yT_d = nc.dram_tensor("yT", [C, N], f32, kind="ExternalOutput")
    s_d = nc.dram_tensor("s_scr", [NH, N], f16)
    inv_d = nc.dram_tensor("inv_scr", [NH, N], f16)

    with tile.TileContext(nc) as tc:
        with (
            tc.tile_pool(name="cst", bufs=1) as cst,
            tc.tile_pool(name="bias_pool", bufs=2) as bias_pool,
            tc.tile_pool(name="pt_pool", bufs=2) as pt_pool,
            tc.tile_pool(name="ps_s", bufs=2, space="PSUM") as ps_s,
            tc.tile_pool(name="ps_o", bufs=2, space="PSUM") as ps_o,
            tc.tile_pool(name="ps_w", bufs=2, space="PSUM") as ps_w,
        ):
            # ---- persistent SBUF ----
            q_t = cst.tile([P, CT, N], f16)          # Q^T  [c, t]
            k_pad = cst.tile([P, NH, N], f16)        # zero-padded K^T per head
            v_aug = cst.tile([P, KT, NH, HD + 1], f16)  # V + ones column
            attn = cst.tile([P, CT, N], f16)         # normalized attn out ^T
            acc = cst.tile([P, CT, N], f16)          # proj partial (k0..3)+pb
            xT = cst.tile([P, CT, N], f16)
            wv = cst.tile([P, CT, C], f16)
            wqk = cst.tile([P, CT, 2 * C], f16)
            wp = cst.tile([P, CT, C], f16)
            qkb = cst.tile([P, OT_QK], f32)
            vb_bc = cst.tile([P, C], f16)
            pbias = cst.tile([P, CT], f32)

            biasT = {}

            def load_bias(h):
                biasT[h] = bias_pool.tile([P, KT, N], f16, tag="biasT",
                                          name=f"biasT{h}")
                nc.sync.dma_start(
                    biasT[h], biasT_d[h].rearrange("(kt p) q -> p kt q", p=P))

            # ---- input DMAs, priority order ----
            xT_src = xT_d[:].rearrange("(k p) t -> p k t", p=P)
            wv_src = wv_d[:].rearrange("(k p) o -> p k o", p=P)
            wqk_src = wqk_d[:].rearrange("(k p) o -> p k o", p=P)
            wp_src = wp_d[:].rearrange("(k p) o -> p k o", p=P)
            for k in range(CT):
                nc.sync.dma_start(xT[:, k, :], xT_src[:, k, :])
                nc.sync.dma_start(wv[:, k, :], wv_src[:, k, :])
            nc.sync.dma_start(vb_bc, bass.AP(tensor=vb_d, offset=0,
                                             ap=[[0, P], [1, C]]))
            nc.sync.dma_start(qkb, qkb_d[:].rearrange("j p -> p j"))
            for k in range(CT):
                nc.sync.dma_start(wqk[:, k, :], wqk_src[:, k, :])
            load_bias(0)
            load_bias(1)
            for k in range(CT):
                nc.sync.dma_start(wp[:, k, :], wp_src[:, k, :])
            nc.sync.dma_start(pbias, pb_d[:].rearrange("j p -> p j"))

            # preload the {ln, exp} activation table set once so the
            # per-head inv = exp(-ln(s)) chain never thrashes ACT tables
            # (set 6 = natural_log_exp_and_others in act_info.json)
            nc.scalar.add_instruction(mybir.InstLoadActFuncSet(
                name="preload_ln_exp", act_func_set_id=6, ins=[], outs=[]))

            # one-time fills: ones column on DVE (tiny; gpsimd memsets take
            # ~12us and the V evictions WAW-wait on them), k_pad zeros on
            # GPSIMD (first consumed by segment-0 S matmuls, plenty late)
            nc.vector.memset(v_aug[:, :, :, HD:HD + 1], 1.0)
            nc.gpsimd.memset(k_pad[64:128, 0:NH:2, :], 0.0)
            nc.gpsimd.memset(k_pad[0:64, 1:NH:2, :], 0.0)

            # ---- emission helpers ----
            def v_units(tt):
                # V natural [t, o']: lhsT = xT tile (shared across vc)
                pvs = [ps_w.tile([P, QN], f32, tag="pw", name=f"pv{vc}")
                       for vc in range(VC)]
                for k in range(CT):
                    for vc in range(VC):
                        nc.tensor.matmul(
                            pvs[vc][:, 0:VN], xT[:, k, tt * P:(tt + 1) * P],
                            wv[:, k, vc * VN:(vc + 1) * VN],
                            start=(k == 0), stop=(k == CT - 1))
                    yield
                for vc in range(VC):
                    h0 = vc * (NH // VC)
                    nc.vector.tensor_add(
                        v_aug[:, tt, h0:h0 + NH // VC, 0:HD],
                        pvs[vc][:, 0:VN], vb_bc[:, vc * VN:(vc + 1) * VN])
                yield

            def qk_units(j):
                # (jj, k) matmul units for the QK projection tile pair j
                for jj in (j, CT + j):
                    pqs = [ps_w.tile([P, QN], f32, tag="pw", name=f"pq{qc}")
                           for qc in range(QC)]
                    for k in range(CT):
                        for qc in range(QC):
                            nc.tensor.matmul(
                                pqs[qc], wqk[:, k, jj * P:(jj + 1) * P],
                                xT[:, k, qc * QN:(qc + 1) * QN],
                                start=(k == 0), stop=(k == CT - 1))
                        yield
                    # evictions on ACT (Identity + per-partition bias AP;
                    # set 6 contains identity so no table thrash) - DVE is
                    # the tighter engine in-segment
                    for qc in range(QC):
                        if jj < CT:
                            nc.scalar.activation(
                                q_t[:, jj, qc * QN:(qc + 1) * QN], pqs[qc],
                                AF.Identity, bias=qkb[:, jj:jj + 1],
                                scale=1.0)
                        else:
                            h0 = 2 * (jj - CT)
                            nc.scalar.activation(
                                k_pad[0:64, h0, qc * QN:(qc + 1) * QN],
                                pqs[qc][0:64, :], AF.Identity,
                                bias=qkb[0:64, jj:jj + 1], scale=1.0)
                            nc.scalar.activation(
                                k_pad[64:128, h0 + 1, qc * QN:(qc + 1) * QN],
                                pqs[qc][64:128, :], AF.Identity,
                                bias=qkb[64:128, jj:jj + 1], scale=1.0)
                    yield

            def proj_partial_units(j):
                # proj tile j partial: contraction k=0..3 (those attn
                # c-tiles are normalized by segment 9), +pb on eviction
                # into the f32 SBUF accumulator. Tail adds k4,k5.
                pys = [ps_w.tile([P, QN], f32, tag="pw", name=f"py{qc}")
                       for qc in range(QC)]
                for k in range(4):
                    for qc in range(QC):
                        nc.tensor.matmul(
                            pys[qc], wp[:, k, j * P:(j + 1) * P],
                            attn[:, k, qc * QN:(qc + 1) * QN],
                            start=(k == 0), stop=(k == 3))
                    yield
                for qc in range(QC):
                    nc.scalar.activation(
                        acc[:, j, qc * QN:(qc + 1) * QN], pys[qc],
                        AF.Identity, bias=pbias[:, j:j + 1], scale=1.0)
                yield

            # ---- denominator chain, split in 3 parts so the in-order
            # ACT/DVE queues never block segment work behind DMA waits ----
            def chain_part1(h, po_t):
                # DVE: denom row out of PSUM; DMA to DRAM; DMA reshape back
                # as [128,8] (partition p holds tokens 8p..8p+7)
                s_sb = pt_pool.tile([1, QC, QN], f16, tag="s_sb", name="s_sb")
                nc.vector.tensor_scalar_add(s_sb, po_t[HD:HD + 1, :, :], 0.0)
                w1 = nc.sync.dma_start(s_d[h], s_sb)
                s128 = pt_pool.tile([P, RT], f16, tag="s128", name="s128")
                r1 = nc.sync.dma_start(
                    s128, bass.AP(tensor=s_d, offset=h * N,
                                  ap=[[RT, P], [1, RT]]))
                _add_dep_helper(r1.ins, w1.ins, sync=True, reason="s RAW")
                return s128

            def chain_part2(h, s128):
                # ACT: inv = exp(-ln(s)) on [128,8] (~190ns each vs 853ns
                # on the [1,1024] layout); DMA out linear; broadcast read
                nc.scalar.activation(s128, s128, AF.Ln, bias=0.0, scale=1.0)
                nc.scalar.activation(s128, s128, AF.Exp, bias=0.0, scale=-1.0)
                w2 = nc.sync.dma_start(
                    bass.AP(tensor=inv_d, offset=h * N,
                            ap=[[RT, P], [1, RT]]), s128)
                inv_bc = pt_pool.tile([HD, N], f16, tag="invbc", name="invbc")
                r2 = nc.sync.dma_start(
                    inv_bc, bass.AP(tensor=inv_d, offset=h * N,
                                    ap=[[0, HD], [1, N]]))
                _add_dep_helper(r2.ins, w2.ins, sync=True, reason="inv RAW")
                return inv_bc

            def chain_part3(h, po_t, inv_bc):
                # fused normalize+evict: attn = (po * 1) * inv_bc
                pbase = (h % 2) * 64
                j = h // 2
                for qc in range(QC):
                    nc.vector.scalar_tensor_tensor(
                        attn[pbase:pbase + HD, j, qc * QN:(qc + 1) * QN],
                        po_t[0:HD, qc, :], 1.0,
                        inv_bc[0:HD, qc * QN:(qc + 1) * QN],
                        MUL, MUL)

            def emit_chain_direct(h, po_t):
                # V14-style low-latency chain for the tail heads: ln/exp on
                # the [1,1024] psum row directly (ACT is idle at the tail;
                # one DMA round trip instead of two -> ~3.5us vs ~8.5us)
                inv_t = pt_pool.tile([1, QC, QN], f16, tag="s_sb",
                                     name="inv_t")
                nc.scalar.activation(inv_t, po_t[HD:HD + 1, :, :],
                                     AF.Ln, bias=0.0, scale=1.0)
                nc.scalar.activation(inv_t, inv_t,
                                     AF.Exp, bias=0.0, scale=-1.0)
                w = nc.sync.dma_start(inv_d[h], inv_t)
                inv_bc = pt_pool.tile([HD, N], f16, tag="invbc",
                                      name="invbc")
                r = nc.sync.dma_start(
                    inv_bc, bass.AP(tensor=inv_d, offset=h * N,
                                    ap=[[0, HD], [1, N]]))
                _add_dep_helper(r.ins, w.ins, sync=True, reason="inv RAW")
                chain_part3(h, po_t, inv_bc)

            # ---- pre-block: all of V, then QK tile pair 0 ----
            for tt in range(KT):
                for _ in v_units(tt):
                    pass
            for _ in qk_units(0):
                pass

            # ---- sliding head pipeline ----
            # Per head h: the S/exp/bias-mult stream for h runs with the
            # previous head's PV matmuls (staggered one k-tile), one unit
            # of the QK projection stream per k-tile (segments 0-9; two
            # proj-partial units per k-tile in segments 10-11), and the
            # chain for head h-2 woven in after kt slots 0/2/4.
            stream_qk = itertools.chain(*[qk_units(j) for j in range(1, NP)])
            stream_proj = itertools.chain(
                *[proj_partial_units(j) for j in range(CT)])
            po = {}
            pt = {}
            chain_state = {}
            for h in range(NH):
                pt[h] = pt_pool.tile([P, KT, N], f16, tag="pt",
                                     name=f"pt{h}")
                if h >= 1:
                    po[h - 1] = ps_o.tile([HD + 1, QC, QN], f32, tag="po",
                                          name=f"po{h - 1}")

                def pv_mms(hh, kt):
                    for qc in range(QC):
                        nc.tensor.matmul(
                            po[hh][:, qc, :], v_aug[:, kt, hh, :],
                            pt[hh][:, kt, qc * QN:(qc + 1) * QN],
                            start=(kt == 0), stop=(kt == KT - 1))

                for kt in range(KT):
                    for qc in range(QC):
                        psq = ps_s.tile([P, QN], f32, tag="pss", name="pss")
                        nc.tensor.matmul(
                            psq,
                            k_pad[:, h, kt * P:(kt + 1) * P],
                            q_t[:, h // 2, qc * QN:(qc + 1) * QN],
                            start=True, stop=True)
                        nc.scalar.activation(
                            pt[h][:, kt, qc * QN:(qc + 1) * QN], psq,
                            AF.Exp, bias=0.0, scale=1.0)
                    if h < 10:
                        next(stream_qk, None)
                    else:
                        next(stream_proj, None)
                        next(stream_proj, None)
                    # PV fillers staggered one k-tile behind the S stream
                    if h >= 1 and kt >= 1:
                        pv_mms(h - 1, kt - 1)
                    # gpsimd (Pool) takes kt 0/1: inputs ready earliest in
                    # the segment (engine rule), and biasT[h]'s last reader
                    # becomes the DVE kt7 mult, so load_bias(h+2)'s buffer
                    # WAR resolves at segment end instead of a gpsimd-lag
                    # later (that wait convoyed the whole Sync queue in V15)
                    if kt in (0, 1):
                        nc.gpsimd.tensor_mul(pt[h][:, kt, :], pt[h][:, kt, :],
                                             biasT[h][:, kt, :])
                    else:
                        for qc in range(QC):
                            nc.vector.tensor_mul(
                                pt[h][:, kt, qc * QN:(qc + 1) * QN],
                                pt[h][:, kt, qc * QN:(qc + 1) * QN],
                                biasT[h][:, kt, qc * QN:(qc + 1) * QN])
                    # weave the h-2 chain into this segment after slots
                    # 0/3/5: part1's DMAs are in flight well before part2's
                    # ACT ops, part2's before part3's STT, so the in-order
                    # ACT/DVE queues never block segment work on DMA waits
                    if h >= 2:
                        if kt == 0:
                            chain_state[h - 2] = [
                                chain_part1(h - 2, po[h - 2])]
                        elif kt == 3:
                            chain_state[h - 2].append(
                                chain_part2(h - 2, chain_state[h - 2][0]))
                        elif kt == 5:
                            chain_part3(h - 2, po[h - 2],
                                        chain_state[h - 2][1])
                            del chain_state[h - 2]
                if h >= 1:
                    pv_mms(h - 1, KT - 1)
                # bias bulk DMA for h+2 at segment end: behind this
                # segment's chain DMAs on the Sync queue, and its buffer
                # WAR (DVE kt7 mult just emitted) resolves ~immediately
                if h + 2 < NH:
                    load_bias(h + 2)
                if h >= 2:
                    del po[h - 2], pt[h - 2]

            # ---- tail: last chains, last PV, proj k4/k5 groups ----
            emit_chain_direct(NH - 2, po[NH - 2])
            po[NH - 1] = ps_o.tile([HD + 1, QC, QN], f32, tag="po",
                                   name=f"po{NH - 1}")
            for kt in range(KT):
                for qc in range(QC):
                    nc.tensor.matmul(
                        po[NH - 1][:, qc, :], v_aug[:, kt, NH - 1, :],
                        pt[NH - 1][:, kt, qc * QN:(qc + 1) * QN],
                        start=(kt == 0), stop=(kt == KT - 1))
            emit_chain_direct(NH - 1, po[NH - 1])

            # per-j (k4,k5) groups: k4 runs during chain(11)'s DMA
            # latency, k5 after STT(10)/STT(11) write attn ct5. ps_w and
            # ps_s pools alternate so j0/j1's k4 pre-run while groups
            # stay open across the STT wait.
            yT_dst = yT_d[:].rearrange("(j p) t -> p j t", p=P)
            for j in range(CT):
                pool = ps_w if j % 2 == 0 else ps_s
                pys = [pool.tile([P, QN], f32, tag="pw" if j % 2 == 0
                                 else "pss", name=f"pt{qc}")
                       for qc in range(QC)]
                for k in (4, 5):
                    for qc in range(QC):
                        nc.tensor.matmul(
                            pys[qc], wp[:, k, j * P:(j + 1) * P],
                            attn[:, k, qc * QN:(qc + 1) * QN],
                            start=(k == 4), stop=(k == 5))
                for qc in range(QC):
                    yb = cst.tile([P, QN], f32, tag="yb", bufs=4, name="yb")
                    nc.vector.scalar_tensor_tensor(
                        yb, pys[qc], 1.0,
                        acc[:, j, qc * QN:(qc + 1) * QN], MUL, ADD)
                    nc.sync.dma_start(
                        yT_dst[:, j, qc * QN:(qc + 1) * QN], yb)

    nc.compile()
    return nc


def _get_nc():
    if "nc" not in _CACHE:
        _CACHE["nc"] = _build()
    return _CACHE["nc"]


def prepare_inputs(x, qkv_w, q_bias, v_bias, proj_w, proj_b, rel_table,
                   rel_index):
    """Host-side resharding/layout prep. Returns per-core input maps."""
    scale = HD ** -0.5
    x = np.asarray(x, np.float32)
    qkv_w = np.asarray(qkv_w, np.float32)
    q_bias = np.asarray(q_bias, np.float32)
    v_bias = np.asarray(v_bias, np.float32)
    proj_w = np.asarray(proj_w, np.float32)
    proj_b = np.asarray(proj_b, np.float32)
    rel_table = np.asarray(rel_table, np.float32)
    rel_index = np.asarray(rel_index)

    wq = qkv_w[0:C, :] * scale          # [o, c] rows scaled
    wk = qkv_w[C:2 * C, :]
    wv_ = qkv_w[2 * C:3 * C, :]
    wqk = np.ascontiguousarray(
        np.concatenate([wq, wk], axis=0).T).astype(np.float16)   # [c, 2C]
    wv_t = np.ascontiguousarray(wv_.T).astype(np.float16)        # [c, C]
    wp = np.ascontiguousarray(proj_w.T).astype(np.float16)       # [c, co]
    qkb = np.concatenate([q_bias * scale, np.zeros(C, np.float32)])
    qkb = np.ascontiguousarray(qkb.reshape(OT_QK, P))
    pb = np.ascontiguousarray(proj_b.reshape(CT, P))

    # bias[q, k, h] = rel_table[rel_index[q, k]]; ship exp(biasT[h, k, q])
    # so the kernel folds the softmax bias multiplicatively into P^T
    bias = rel_table[rel_index.reshape(-1)].reshape(N, N, NH)
    biasT = np.ascontiguousarray(
        np.exp(bias.transpose(2, 1, 0), dtype=np.float32)).astype(np.float16)

    shared = {
        "wqk": wqk, "wv": wv_t, "wp": wp, "qkb": qkb,
        "vb": v_bias.astype(np.float16), "pb": pb, "biasT": biasT,
    }
    in_maps = []
    for b in range(B):
        xt = np.ascontiguousarray(
            x[b].reshape(N, C).T).astype(np.float16)
        in_maps.append({"xT": xt, **shared})
    return in_maps


def kernel(x, qkv_w, q_bias, v_bias, proj_w, proj_b, rel_table, rel_index,
           _trace=False):
    from concourse.bass_utils import run_bass_kernel_spmd

    nc = _get_nc()
    in_maps = prepare_inputs(x, qkv_w, q_bias, v_bias, proj_w, proj_b,
                             rel_table, rel_index)
    kwargs = {}
    if _trace:
        import concourse.bass_utils as _bu
        _bu.upload_artifacts = lambda tmpdir: tmpdir
        kwargs = {"trace": True}
    res = run_bass_kernel_spmd(nc, in_maps, core_ids=list(range(B)), **kwargs)
    out = np.empty((B, WS, WS, C), np.float32)
    for b in range(B):
        out[b] = res.results[b]["yT"].T.reshape(WS, WS, C)
    if _trace:
        _CACHE["last_result"] = res
    return out
